# revision 6
# baseline (speedup 1.0000x reference)
"""HGT (2-type, 3-edge-type, 2-layer) Trainium2 kernel.

Sharding: destination nodes are partitioned across the 8 cores; every core
replicates the dense projections (q and fused relation K/V tables) and
processes only edges whose destination it owns, so no collectives are needed.
Segment softmax + scatter-add are done with one-hot matmuls on the PE array;
source-side features are fetched with indirect (gather) DMAs.
The per-layer program is compiled once and executed twice (layer weights and
activations are just data); the host performs the layer-boundary
concat/transpose of activations and the final tiny graph-mean + output matmul.
"""
import sys
sys.path.insert(0, '/opt/trn_rl_repo')
import numpy as np

import concourse.bass as bass
import concourse.bacc as bacc
import concourse.mybir as mybir
import concourse.tile as tile
from concourse.masks import make_identity
from concourse.bass_utils import run_bass_kernel_spmd

P = 128
NP_, NA_ = 100000, 50000
C, H, L, G, OUT = 128, 8, 2, 64, 64
D = C // H
SQRT_D = float(np.sqrt(D))
NCORES = 8
OWN_P, OWN_A = NP_ // NCORES, NA_ // NCORES          # 12500 / 6250
NT_P, NT_A = (OWN_P + P - 1) // P, (OWN_A + P - 1) // P  # 98 / 49 tiles per core
PAD_P, PAD_A = NT_P * P, NT_A * P                    # 12544 / 6272
NPf, NAf = NCORES * PAD_P, NCORES * PAD_A            # 100352 / 50176

# (name, src_type, dst_type): 0=paper, 1=author
ETYPES = [("pp", 0, 0), ("ap", 1, 0), ("pa", 0, 1)]
F32 = mybir.dt.float32
I32 = mybir.dt.int32

_cache = {}


def _build(cpts):
    """One generic HGT layer, SPMD across 8 cores (identical program,
    per-core data). cpts = dict etype-name -> chunks-per-dst-tile."""
    nc = bacc.Bacc(None, target_bir_lowering=False)

    xpT = nc.dram_tensor("xpT", [C, NPf], F32, kind="ExternalInput")
    xaT = nc.dram_tensor("xaT", [C, NAf], F32, kind="ExternalInput")
    xpoT = nc.dram_tensor("xpoT", [C, PAD_P], F32, kind="ExternalInput")
    xaoT = nc.dram_tensor("xaoT", [C, PAD_A], F32, kind="ExternalInput")
    xpo = nc.dram_tensor("xpo", [PAD_P, C], F32, kind="ExternalInput")
    xao = nc.dram_tensor("xao", [PAD_A, C], F32, kind="ExternalInput")
    Wq = nc.dram_tensor("Wq", [2, C, C], F32, kind="ExternalInput")
    Wkvp = nc.dram_tensor("Wkvp", [C, 4 * C], F32, kind="ExternalInput")  # pp|pa
    Wkva = nc.dram_tensor("Wkva", [C, 2 * C], F32, kind="ExternalInput")  # ap
    Wa = nc.dram_tensor("Wa", [2, C, C], F32, kind="ExternalInput")
    ed = {}
    for e, st, dt in ETYPES:
        nt = NT_P if dt == 0 else NT_A
        ed[e] = (
            nc.dram_tensor(f"dl_{e}", [nt, P, cpts[e]], F32, kind="ExternalInput"),
            nc.dram_tensor(f"si_{e}", [nt, P, cpts[e]], I32, kind="ExternalInput"),
        )
    btp = nc.dram_tensor("btp", [P, NT_P], F32, kind="ExternalInput")
    bta = nc.dram_tensor("bta", [P, NT_A], F32, kind="ExternalInput")
    oxp = nc.dram_tensor("oxp", [PAD_P, C], F32, kind="ExternalOutput")
    oxa = nc.dram_tensor("oxa", [PAD_A, C], F32, kind="ExternalOutput")
    poolp = nc.dram_tensor("poolp", [G, C], F32, kind="ExternalOutput")
    poola = nc.dram_tensor("poola", [G, C], F32, kind="ExternalOutput")

    with tile.TileContext(nc) as tc:
        with tc.tile_pool(name="cst", bufs=1) as cst, \
             tc.tile_pool(name="qtp", bufs=1) as qtp, \
             tc.tile_pool(name="ld", bufs=3) as ld, \
             tc.tile_pool(name="wk", bufs=3) as wk, \
             tc.tile_pool(name="ps", bufs=3, space="PSUM") as ps, \
             tc.tile_pool(name="agp", bufs=3, space="PSUM") as agp, \
             tc.tile_pool(name="plp", bufs=1, space="PSUM") as plp, \
             tc.tile_pool(name="dr", bufs=1, space="DRAM") as dr:

            ident = cst.tile([P, P], F32)
            make_identity(nc, ident[:])
            iota_i = cst.tile([P, P], I32)
            nc.gpsimd.iota(iota_i[:], pattern=[[1, P]], base=0, channel_multiplier=0)
            iota_r = cst.tile([P, P], F32)
            nc.vector.tensor_copy(iota_r[:], iota_i[:])

            # weights resident in SBUF
            w_q = [cst.tile([C, C], F32, tag=f"wq{t}", name=f"wq{t}") for t in range(2)]
            for t in range(2):
                nc.sync.dma_start(w_q[t][:], Wq[t])
            w_kvp = cst.tile([C, 4 * C], F32)
            nc.sync.dma_start(w_kvp[:], Wkvp[:])
            w_kva = cst.tile([C, 2 * C], F32)
            nc.sync.dma_start(w_kva[:], Wkva[:])
            w_a = [cst.tile([C, C], F32, tag=f"wa{t}", name=f"wa{t}") for t in range(2)]
            for t in range(2):
                nc.sync.dma_start(w_a[t][:], Wa[t])
            t_btp = cst.tile([P, NT_P], F32)
            nc.sync.dma_start(t_btp[:], btp[:])
            t_bta = cst.tile([P, NT_A], F32)
            nc.sync.dma_start(t_bta[:], bta[:])

            # ---- relation K/V tables (node-major, DRAM) -------------------
            kvt = {"pp": dr.tile([NPf, 2 * C], F32, tag="kvpp", name="kvpp"),
                   "pa": dr.tile([NPf, 2 * C], F32, tag="kvpa", name="kvpa"),
                   "ap": dr.tile([NAf, 2 * C], F32, tag="kvap", name="kvap")}
            for src, xt, n_full in ((0, xpT, NPf), (1, xaT, NAf)):
                wt = w_kvp if src == 0 else w_kva
                ncols = 4 * C if src == 0 else 2 * C
                for g in range(n_full // P):
                    xg = ld.tile([C, P], F32, tag="xg")
                    nc.sync.dma_start(xg[:], xt[:, g * P:(g + 1) * P])
                    kp = ps.tile([P, ncols], F32, tag="mm", space="PSUM")
                    nc.tensor.matmul(out=kp[:], lhsT=xg[:], rhs=wt[:],
                                     start=True, stop=True)
                    ks = wk.tile([P, ncols], F32, tag="kvsb")
                    if g % 2 == 0:
                        nc.scalar.activation(out=ks[:], in_=kp[:],
                                             func=mybir.ActivationFunctionType.Copy)
                    else:
                        nc.vector.tensor_copy(ks[:], kp[:])
                    if src == 0:
                        nc.sync.dma_start(kvt["pp"][g * P:(g + 1) * P, :], ks[:, :2 * C])
                        nc.sync.dma_start(kvt["pa"][g * P:(g + 1) * P, :], ks[:, 2 * C:])
                    else:
                        nc.sync.dma_start(kvt["ap"][g * P:(g + 1) * P, :], ks[:])

            # ---- q tiles for owned dst nodes (SBUF-resident) --------------
            qt = {0: [], 1: []}
            for t, xot, nt in ((0, xpoT, NT_P), (1, xaoT, NT_A)):
                for i in range(nt):
                    xg = ld.tile([C, P], F32, tag="xg")
                    nc.sync.dma_start(xg[:], xot[:, i * P:(i + 1) * P])
                    qp = ps.tile([P, C], F32, tag="mm", space="PSUM")
                    nc.tensor.matmul(out=qp[:], lhsT=xg[:], rhs=w_q[t][:],
                                     start=True, stop=True)
                    q_sb = qtp.tile([P, C], F32, tag=f"q{t}_{i}", name=f"q{t}_{i}")
                    nc.scalar.activation(out=q_sb[:], in_=qp[:],
                                         func=mybir.ActivationFunctionType.Copy)
                    qt[t].append(q_sb)

            # ---- edge aggregation + post per dst tile ---------------------
            for t, (nt, xown, xownT_unused, oxt, bt, poolt) in enumerate((
                    (NT_P, xpo, xpoT, oxp, t_btp, poolp),
                    (NT_A, xao, xaoT, oxa, t_bta, poola))):
                etl = [z for z in ETYPES if z[2] == t]
                pool_ps = plp.tile([G, C], F32, tag=f"pool{t}", space="PSUM")
                for i in range(nt):
                    aggs = []
                    for e, st, dt in etl:
                        cpt = cpts[e]
                        dl_t = ld.tile([P, cpt], F32, tag=f"dl{t}")
                        nc.sync.dma_start(dl_t[:], ed[e][0][i])
                        si_t = ld.tile([P, cpt], I32, tag=f"si{t}")
                        nc.sync.dma_start(si_t[:], ed[e][1][i])
                        agg = agp.tile([P, 136], F32, tag="agg", space="PSUM")
                        for c in range(cpt):
                            kvg = wk.tile([P, 2 * C], F32, tag="kvg")
                            nc.gpsimd.indirect_dma_start(
                                out=kvg[:], out_offset=None, in_=kvt[e][:],
                                in_offset=bass.IndirectOffsetOnAxis(
                                    ap=si_t[:, c:c + 1], axis=0))
                            t_S = wk.tile([P, P], F32, tag="S")
                            nc.vector.tensor_tensor(
                                out=t_S[:], in0=dl_t[:, c:c + 1].to_broadcast([P, P]),
                                in1=iota_r[:], op=mybir.AluOpType.is_equal)
                            tp = ps.tile([P, P], F32, tag="mm", space="PSUM")
                            nc.tensor.transpose(out=tp[:], in_=t_S[:], identity=ident[:])
                            t_T = wk.tile([P, P], F32, tag="T")
                            nc.scalar.activation(out=t_T[:], in_=tp[:],
                                                 func=mybir.ActivationFunctionType.Copy)
                            qe = ps.tile([P, P], F32, tag="mm", space="PSUM")
                            nc.tensor.matmul(out=qe[:], lhsT=t_T[:], rhs=qt[t][i][:],
                                             start=True, stop=True)
                            qk = wk.tile([P, P], F32, tag="qk")
                            nc.vector.tensor_tensor(out=qk[:], in0=qe[:],
                                                    in1=kvg[:, 0:C],
                                                    op=mybir.AluOpType.mult)
                            exv = wk.tile([P, 136], F32, tag="exv")
                            nc.vector.tensor_reduce(
                                out=exv[:, C:C + H],
                                in_=qk[:].rearrange("p (h d) -> p h d", h=H),
                                axis=mybir.AxisListType.X, op=mybir.AluOpType.add)
                            nc.scalar.activation(out=exv[:, C:C + H], in_=exv[:, C:C + H],
                                                 func=mybir.ActivationFunctionType.Exp)
                            nc.vector.tensor_tensor(
                                out=exv[:, 0:C].rearrange("p (h d) -> p h d", h=H),
                                in0=kvg[:, C:2 * C].rearrange("p (h d) -> p h d", h=H),
                                in1=exv[:, C:C + H].broadcast_to([P, H, D]),
                                op=mybir.AluOpType.mult)
                            nc.tensor.matmul(out=agg[:], lhsT=t_S[:], rhs=exv[:],
                                             start=(c == 0), stop=(c == cpt - 1))
                        aggs.append(agg)
                    # normalize + combine
                    att = wk.tile([P, C], F32, tag="att")
                    for k, agg in enumerate(aggs):
                        dn = wk.tile([P, H], F32, tag="dn")
                        nc.vector.tensor_scalar_add(dn[:], agg[:, C:C + H], 1e-20)
                        rc = wk.tile([P, H], F32, tag="rc")
                        nc.vector.reciprocal(rc[:], dn[:])
                        if k == 0:
                            nc.vector.tensor_tensor(
                                out=att[:].rearrange("p (h d) -> p h d", h=H),
                                in0=agg[:, 0:C].rearrange("p (h d) -> p h d", h=H),
                                in1=rc[:].broadcast_to([P, H, D]),
                                op=mybir.AluOpType.mult)
                        else:
                            att2 = wk.tile([P, C], F32, tag="att2")
                            nc.vector.tensor_tensor(
                                out=att2[:].rearrange("p (h d) -> p h d", h=H),
                                in0=agg[:, 0:C].rearrange("p (h d) -> p h d", h=H),
                                in1=rc[:].broadcast_to([P, H, D]),
                                op=mybir.AluOpType.mult)
                            nc.vector.tensor_tensor(out=att[:], in0=att[:], in1=att2[:],
                                                    op=mybir.AluOpType.add)
                    gl = wk.tile([P, C], F32, tag="gl")
                    nc.scalar.activation(out=gl[:], in_=att[:],
                                         func=mybir.ActivationFunctionType.Gelu)
                    gt_ps = ps.tile([P, P], F32, tag="mm", space="PSUM")
                    nc.tensor.transpose(out=gt_ps[:], in_=gl[:], identity=ident[:])
                    gt = wk.tile([P, C], F32, tag="gt")
                    nc.scalar.activation(out=gt[:], in_=gt_ps[:],
                                         func=mybir.ActivationFunctionType.Copy)
                    ao_ps = ps.tile([P, C], F32, tag="mm", space="PSUM")
                    nc.tensor.matmul(out=ao_ps[:], lhsT=gt[:], rhs=w_a[t][:],
                                     start=True, stop=True)
                    xo_t = ld.tile([P, C], F32, tag="xo")
                    nc.sync.dma_start(xo_t[:], xown[i * P:(i + 1) * P, :])
                    nx = wk.tile([P, C], F32, tag="nx")
                    nc.vector.tensor_tensor(out=nx[:], in0=xo_t[:], in1=ao_ps[:],
                                            op=mybir.AluOpType.add)
                    nc.sync.dma_start(oxt[i * P:(i + 1) * P, :], nx[:])
                    # graph pooling (segment-sum by batch id via one-hot matmul)
                    sg = wk.tile([P, G], F32, tag="sg")
                    nc.vector.tensor_tensor(out=sg[:],
                                            in0=bt[:, i:i + 1].to_broadcast([P, G]),
                                            in1=iota_r[:, 0:G],
                                            op=mybir.AluOpType.is_equal)
                    nc.tensor.matmul(out=pool_ps[:], lhsT=sg[:], rhs=nx[:],
                                     start=(i == 0), stop=(i == nt - 1))
                pool_sb = wk.tile([G, C], F32, tag="poolsb")
                nc.vector.tensor_copy(pool_sb[:], pool_ps[:])
                nc.sync.dma_start(poolt[:], pool_sb[:])
    if not nc.is_finalized():
        nc.finalize()
    return nc


def _shard_edges(src, dst, own, nt, n_src_real):
    """Per-core (dstl f32 [nt,P,cpt_needed-major], srci) arrays; returns list
    of (dstl, srci) before cpt-padding plus per-core needed cpt."""
    out = []
    for i in range(NCORES):
        lo = i * own
        sel = (dst >= lo) & (dst < lo + own)
        dl = (dst[sel] - lo).astype(np.int64)
        ss = src[sel].astype(np.int64)
        order = np.argsort(dl, kind="stable")
        dl = dl[order]; ss = ss[order]
        tid = dl >> 7
        counts = np.bincount(tid, minlength=nt)
        starts = np.concatenate(([0], np.cumsum(counts)))[:nt]
        rank = np.arange(len(dl)) - starts[tid]
        cpt = int((counts.max() + P - 1) // P) if len(dl) else 1
        out.append((dl, ss, tid, rank, cpt))
    return out


def _pack_edges(shards, nt, cpt):
    res = []
    for dl, ss, tid, rank, _ in shards:
        dstl = np.full((nt, P, cpt), 999.0, np.float32)
        srci = np.zeros((nt, P, cpt), np.int32)
        flat = tid * (P * cpt) + (rank % P) * cpt + (rank // P)
        dstl.reshape(-1)[flat] = (dl - tid * P).astype(np.float32)
        srci.reshape(-1)[flat] = ss.astype(np.int32)
        res.append((dstl, srci))
    return res


def _padT(x, n_pad):
    """[N, C] -> transposed, padded [C, n_pad] f32 contiguous."""
    out = np.zeros((C, n_pad), np.float32)
    out[:, :x.shape[0]] = x.T
    return out


def _pad(x, n_pad):
    out = np.zeros((n_pad, C), np.float32)
    out[:x.shape[0]] = x
    return out


def kernel(**inputs):
    inp = {k: np.asarray(v) for k, v in inputs.items()}
    x_paper = inp["x_paper"].astype(np.float32)
    x_author = inp["x_author"].astype(np.float32)
    Wlin = inp["Wlin"]; Wk = inp["Wk"]; Wq = inp["Wq"]; Wv = inp["Wv"]
    a_rel = inp["a_rel"]; m_rel = inp["m_rel"]; p_rel = inp["p_rel"]
    Wa = inp["Wa"]; skip = inp["skip"]
    Wout = inp["Wout"]; bout = inp["bout"]
    blin = inp["blin"]; bk = inp["bk"]; bq = inp["bq"]; bv = inp["bv"]; ba = inp["ba"]

    # ---- host: fold relation tensors into projection weights -------------
    # k_rel = (x@Wk) @ blockdiag(a_rel*p_rel/sqrt(D)); v_rel = (x@Wv) @ blockdiag(m_rel)
    def blockdiag(M):  # [H, D, D] -> [C, C]
        out = np.zeros((C, C), np.float32)
        for h in range(H):
            out[h * D:(h + 1) * D, h * D:(h + 1) * D] = M[h]
        return out

    W_kv = np.zeros((L, 3, C, 2 * C), np.float32)
    for l in range(L):
        for e, (en, st, dt) in enumerate(ETYPES):
            A = blockdiag(a_rel[l, e] * (p_rel[l, e] / SQRT_D)[:, None, None])
            M = blockdiag(m_rel[l, e])
            W_kv[l, e, :, :C] = Wk[l, st] @ A
            W_kv[l, e, :, C:] = Wv[l, st] @ M
    beta = 1.0 / (1.0 + np.exp(-skip.astype(np.float64)))   # sigmoid
    Wa_eff = (beta[:, :, None, None] * Wa).astype(np.float32)
    omb = (1.0 - beta).astype(np.float32).reshape(L, 2, 1)

    # ---- host: edge sharding ---------------------------------------------
    e_in = {"pp": (inp["edge_pp_src"], inp["edge_pp_dst"], OWN_P, NT_P, NP_),
            "ap": (inp["edge_ap_src"], inp["edge_ap_dst"], OWN_A if False else OWN_P, NT_P, NA_),
            "pa": (inp["edge_pa_src"], inp["edge_pa_dst"], OWN_A, NT_A, NP_)}
    # note: own/nt are determined by the *dst* type: pp,ap -> papers; pa -> authors
    shards = {}
    cpts = {}
    for e, (s, d, own, nt, nsr) in e_in.items():
        sh = _shard_edges(np.asarray(s), np.asarray(d), own, nt, nsr)
        shards[e] = sh
        cpts[e] = max(z[4] for z in sh)
    packed = {e: _pack_edges(shards[e], e_in[e][3], cpts[e]) for e in shards}

    # ---- host: batch vectors / counts ------------------------------------
    bp = np.asarray(inp["batch_paper"]).astype(np.int64)
    bauth = np.asarray(inp["batch_author"]).astype(np.int64)
    cnt_p = np.maximum(np.bincount(bp, minlength=G).astype(np.float32), 1.0)
    cnt_a = np.maximum(np.bincount(bauth, minlength=G).astype(np.float32), 1.0)

    def batch_tiles(b, own, nt):
        res = []
        for i in range(NCORES):
            bb = np.full(nt * P, G + 1.0, np.float32)
            bb[:own] = b[i * own:(i + 1) * own].astype(np.float32)
            res.append(bb.reshape(nt, P).T.copy())
        return res
    btp_c = batch_tiles(bp, OWN_P, NT_P)
    bta_c = batch_tiles(bauth, OWN_A, NT_A)

    # ---- program ----------------------------------------------------------
    key = tuple(sorted(cpts.items()))
    if key not in _cache:
        _cache[key] = _build(cpts)
    nc = _cache[key]

    # ---- layer 0 input activations (host: input projection + relu) -------
    xs = [np.maximum(x_paper @ Wlin[0] + blin[0], 0.0),
          np.maximum(x_author @ Wlin[1] + blin[1], 0.0)]

    for l in range(L):
        xpT_full = _padT(xs[0], NPf)
        xaT_full = _padT(xs[1], NAf)
        in_maps = []
        for i in range(NCORES):
            xpoT_i = np.zeros((C, PAD_P), np.float32)
            xpoT_i[:, :OWN_P] = xpT_full[:, i * OWN_P:(i + 1) * OWN_P]
            xaoT_i = np.zeros((C, PAD_A), np.float32)
            xaoT_i[:, :OWN_A] = xaT_full[:, i * OWN_A:(i + 1) * OWN_A]
            m = {
                "xpT": xpT_full, "xaT": xaT_full,
                "xpoT": xpoT_i, "xaoT": xaoT_i,
                "xpo": np.ascontiguousarray(omb[l, 0, 0] * xpoT_i.T),
                "xao": np.ascontiguousarray(omb[l, 1, 0] * xaoT_i.T),
                "Wq": np.ascontiguousarray(Wq[l]),
                "Wkvp": np.ascontiguousarray(
                    np.concatenate([W_kv[l, 0], W_kv[l, 2]], axis=1)),
                "Wkva": np.ascontiguousarray(W_kv[l, 1]),
                "Wa": np.ascontiguousarray(Wa_eff[l]),
                "btp": btp_c[i], "bta": bta_c[i],
            }
            for e in ("pp", "ap", "pa"):
                m[f"dl_{e}"] = packed[e][i][0]
                m[f"si_{e}"] = packed[e][i][1]
            in_maps.append(m)
        res = run_bass_kernel_spmd(nc, in_maps, core_ids=list(range(NCORES)))
        xs = [np.concatenate([res.results[i]["oxp"][:OWN_P] for i in range(NCORES)]),
              np.concatenate([res.results[i]["oxa"][:OWN_A] for i in range(NCORES)])]

    pool_p = np.sum([res.results[i]["poolp"] for i in range(NCORES)], axis=0)
    pool_a = np.sum([res.results[i]["poola"] for i in range(NCORES)], axis=0)
    hg = pool_p / cnt_p[:, None] + pool_a / cnt_a[:, None]
    return (hg @ Wout + bout).astype(np.float32)


# mapping fix for ap dst sizing (dst of ap is papers): own/nt above already use
# papers for pp/ap and authors for pa.
